# revision 29
# baseline (speedup 1.0000x reference)
"""EventTrace kernel for Trainium2 (8 NeuronCores, Bass/Tile).

Computes, for each batch row b:
    ev[t]   = embed[ctrl_tokens[b, t, 1]]          (gather from [64,512] table)
    c[t]    = ALPHA * c[t-1] + ev[t],  c[-1] = prev_trace[b]
    out[b]  = c                                     -> [B, T, D] float32

Algorithm (per core, 2 batch rows):
  Instead of gathering 16 MiB of embeddings, scan *decayed one-hot counts*
  G[v, t] = ALPHA * G[v, t-1] + onehot(idx_t == v) on the vector engine
  (tensor_tensor_scan, both rows in one [128, T] scan), then reconstruct
  each 128-step output block with one K=64 matmul per row:
      C[t, d] = sum_v G[v, t] * embed[v, d]  (+ ALPHA^(t+1) * prev[d])
  The two rows' matmuls use PE row-tiling (tile_position (0,0) / (64,0)) so
  they run concurrently.  The prev-trace carry decays below f32 relevance
  after 128 steps, so it is applied only to block 0 via a fused
  scalar_tensor_tensor during PSUM eviction.

Sharding: batch rows across the 8 cores (2 rows per core); the embedding
table and constants are replicated.
"""

import sys

for _p in ("/root/.axon_site/_ro/trn_rl_repo", "/opt/trn_rl_repo"):
    if _p not in sys.path:
        sys.path.append(_p)

import numpy as np

import concourse.bass as bass
import concourse.tile as tile
from concourse import mybir
from concourse.bass_utils import run_bass_kernel_spmd

ALPHA = 0.9
B, T, V, D = 16, 4096, 64, 512
NCORES = 8
RPC = B // NCORES  # batch rows per core
BLK = 128
NBLK = T // BLK

F32 = mybir.dt.float32
F32R = mybir.dt.float32r

# which engine evicts PSUM for block k (DVE is ~2x faster per copy but also
# runs the scan; ACT is otherwise idle)
def _copy_engine(k):
    return "act" if k % 2 == 0 else "dve"


def build_nc(strip=True):
    nc = bass.Bass(trn_type="TRN2", target_bir_lowering=False)

    # comb: [128, 2+T] f32 — col 0: iota (v index, repeated per row-half),
    # col 1: ALPHA, cols 2..: idx[b] broadcast (row b in partitions b*64..).
    comb_d = nc.dram_tensor("comb", [128, 2 + T], F32, kind="ExternalInput")
    # embed duplicated into both partition halves (pre-rounded to tf32)
    rhs_d = nc.dram_tensor("rhs", [128, D], F32R, kind="ExternalInput")
    # prev_trace[b] broadcast across 128 partitions, one per row
    prev_d = [
        nc.dram_tensor(f"prev{b}", [128, D], F32, kind="ExternalInput")
        for b in range(RPC)
    ]
    # alpha^(p+1) per partition, for the block-0 carry
    apow_d = nc.dram_tensor("apow", [128, 1], F32, kind="ExternalInput")
    out = nc.dram_tensor("out", [RPC, T, D], F32, kind="ExternalOutput")

    with tile.TileContext(nc) as tc:
        with (
            tc.tile_pool(name="const", bufs=1) as cpool,
            tc.tile_pool(name="psum", bufs=8, space="PSUM") as ppool,
            tc.tile_pool(name="outp", bufs=8) as opool,
        ):
            comb_t = cpool.tile([128, 2 + T], F32, name="comb_t")
            nc.gpsimd.dma_start(comb_t[:], comb_d[:, :])
            # rhs flows through a DVE cast: walrus only accepts compute-engine
            # producers for fp32r matmul operands (values must be rounded).
            rhs_stage = cpool.tile([128, D], F32, name="rhs_stage")
            nc.gpsimd.dma_start(rhs_stage[:], rhs_d[:, :])
            rhs_t = cpool.tile([128, D], F32R, name="rhs_t")
            prev_t = [
                cpool.tile([128, D], F32, name=f"prev_t{b}") for b in range(RPC)
            ]
            for b in range(RPC):
                nc.gpsimd.dma_start(prev_t[b][:], prev_d[b][:, :])
            apow_t = cpool.tile([128, 1], F32, name="apow_t")
            nc.gpsimd.dma_start(apow_t[:], apow_d[:, :])

            scr = cpool.tile([128, 8], F32, name="scr")
            nc.vector.memset(scr[:], 0.0)

            # DVE cast (also makes DVE observe the rhs DMA); the tiny copies
            # make DVE observe prev/apow so later consumers need one wait.
            nc.vector.tensor_copy(rhs_t[:], rhs_stage[:])
            nc.vector.tensor_copy(scr[0:1, 1:2], prev_t[0][0:1, 0:1])
            nc.vector.tensor_copy(scr[0:1, 2:3], prev_t[1][0:1, 0:1])
            nc.vector.tensor_copy(scr[0:1, 3:4], apow_t[0:1, 0:1])

            # M[p, t] = 1.0 if idx[p//64, t] == (p % 64) else 0.0
            m2 = cpool.tile([128, T], F32, name="m2")
            nc.vector.tensor_scalar(
                m2[:],
                comb_t[:, 2 : 2 + T],
                comb_t[:, 0:1],
                None,
                mybir.AluOpType.is_equal,
            )
            # G[p, t] = ALPHA * G[p, t-1] + M[p, t]   (both rows at once)
            g2 = cpool.tile([128, T], F32R, name="g2")
            alpha_bc = comb_t[:, 1:2].broadcast_to((128, T))
            nc.vector.tensor_tensor_scan(
                g2[:],
                alpha_bc,
                m2[:],
                0.0,
                mybir.AluOpType.mult,
                mybir.AluOpType.add,
            )

            last_ots = []
            for k in range(NBLK):
                for b in range(RPC):
                    ps = ppool.tile([BLK, D], F32, name="ps")
                    nc.tensor.matmul(
                        ps[:],
                        g2[b * V : (b + 1) * V, k * BLK : (k + 1) * BLK],
                        rhs_t[b * V : (b + 1) * V, :],
                        start=True,
                        stop=True,
                        tile_position=(b * V, 0),
                    )
                    ot = opool.tile([BLK, D], F32, name="ot")
                    wr = "dve" if k == 0 else _copy_engine(k)
                    # 4-byte touch absorbs the WAR wait on this slot's prior
                    # out-DMA, so the eviction op only waits on the matmul.
                    if wr == "act":
                        nc.scalar.copy(ot[0:1, 0:1], scr[0:1, 0:1])
                    else:
                        nc.vector.tensor_copy(ot[0:1, 0:1], scr[0:1, 0:1])
                    if k == 0:
                        # block 0 carries prev_trace: ot = prev*alpha^(p+1) + ps
                        nc.vector.scalar_tensor_tensor(
                            ot[:],
                            prev_t[b][:],
                            apow_t[:, 0:1],
                            ps[:],
                            mybir.AluOpType.mult,
                            mybir.AluOpType.add,
                        )
                    elif _copy_engine(k) == "act":
                        nc.scalar.copy(ot[:], ps[:])
                    else:
                        nc.vector.tensor_copy(ot[:], ps[:])
                    nc.sync.dma_start(out[b, k * BLK : (k + 1) * BLK, :], ot[:])
                    last_ots.append(ot)
                    last_ots = last_ots[-8:]
            # End-of-kernel sinks: touching each of the last 8 output slots
            # makes the DVE stream transitively observe every DMA queue's
            # final completion, so the tail drain needs only one wait after
            # the redundant-wait strip below.
            for ot in last_ots:
                nc.vector.tensor_copy(ot[0:1, 0:1], scr[0:1, 0:1])
    if strip:
        _strip_redundant_waits(nc)
    return nc


def _strip_redundant_waits(nc):
    """Remove statically-implied semaphore waits (vector-clock analysis).

    The TRN2 instruction encodings here accept only ONE sync-wait command
    per instruction, but Tile emits extra waits for pool-slot reuse and the
    kernel-tail drain.  Many of those waits are statically implied by
    program order: engine queues execute in order, each DMA queue completes
    FIFO, and observing a semaphore value inherits every guarantee its
    updaters had.  This pass computes, for every instruction, the semaphore
    floor guaranteed at issue, and drops any wait already implied without
    it.  Straight-line (loop-free) programs only.
    """
    import concourse.mybir as mybir

    insts = []
    for fn in nc.m.functions:
        for bb in fn.blocks:
            for ins in bb.instructions:
                insts.append(ins)

    def waits(ins):
        si = ins.sync_info
        return list(si.on_wait) if si is not None else []

    def updates(ins):
        si = ins.sync_info
        return list(si.on_update) if si is not None else []

    # Streams: compute instructions execute in order per engine; a DMACopy's
    # *data completion* (its sem update) is FIFO per DMA queue, gated by its
    # trigger (engine stream) issue.
    def is_dma(ins):
        return type(ins).__name__ == "InstDMACopy"

    def dma_queue(ins):
        us = updates(ins)
        return us[0].ant_name if us else None

    # sem -> ordered list of (inst_index, add_value); single-updater-stream
    # sems only are used for transitive guarantees.
    sem_updaters = {}
    sem_streams = {}
    for i, ins in enumerate(insts):
        key = ("q", dma_queue(ins)) if is_dma(ins) else ("e", str(ins.engine))
        for u in updates(ins):
            if u.update_mode not in ("sem-inc", "sem-add-imm") or u.update_reg:
                sem_streams.setdefault(u.ant_name, set()).add("reg")
                continue
            sem_updaters.setdefault(u.ant_name, []).append((i, u.update_value))
            sem_streams.setdefault(u.ant_name, set()).add(key)

    single_stream_sems = {s for s, st in sem_streams.items() if len(st) == 1}

    # cumulative sem value right after instruction i's update
    cum_after = {}
    run = {}
    for i, ins in enumerate(insts):
        for u in updates(ins):
            if u.update_mode in ("sem-inc", "sem-add-imm") and not u.update_reg:
                run[u.ant_name] = run.get(u.ant_name, 0) + u.update_value
                cum_after[(i, u.ant_name)] = run[u.ant_name]

    prev_engine = {}
    prev_queue = {}
    last_e = {}
    last_q = {}
    for i, ins in enumerate(insts):
        ek = str(ins.engine)
        prev_engine[i] = last_e.get(ek)
        last_e[ek] = i
        if is_dma(ins):
            qk = dma_queue(ins)
            prev_queue[i] = last_q.get(qk)
            last_q[qk] = i

    n = len(insts)
    # disp[i]: sem floor guaranteed when instruction i dispatches (data-order
    # level).  done[i]: floor when its effects (sem updates) are visible —
    # for a DMACopy that is DATA completion on its queue.
    disp = [dict() for _ in range(n)]
    done = [dict() for _ in range(n)]

    def join_into(dst, src):
        changed = False
        for s, v in src.items():
            if dst.get(s, 0) < v:
                dst[s] = v
                changed = True
        return changed

    def guarantee_of_wait(sem, val):
        """Floor implied by observing sem >= val."""
        out = {sem: val}
        if sem not in single_stream_sems:
            return out
        cum = 0
        for j, add in sem_updaters.get(sem, []):
            cum += add
            join_into(out, done[j])
            if cum >= val:
                break
        return out

    def disp_floor(i, skip_wait=None):
        out = {}
        p = prev_engine[i]
        if p is not None:
            join_into(out, disp[p])
            if not is_dma(insts[p]):
                # same-engine execution is in-order: p's effects precede i's
                join_into(out, done[p])
        for w in waits(insts[i]):
            if w is skip_wait:
                continue
            if w.wait_mode == "sem-ge-imm" and not w.wait_reg:
                join_into(out, guarantee_of_wait(w.ant_name, w.wait_value))
        return out

    def recompute():
        changed = True
        while changed:
            changed = False
            for i, ins in enumerate(insts):
                f = disp_floor(i)
                if join_into(disp[i], f):
                    changed = True
                d = dict(disp[i])
                if is_dma(ins):
                    pq = prev_queue.get(i)
                    if pq is not None:
                        join_into(d, done[pq])
                for u in updates(ins):
                    c = cum_after.get((i, u.ant_name))
                    if c is not None and d.get(u.ant_name, 0) < c:
                        d[u.ant_name] = c
                if join_into(done[i], d):
                    changed = True

    recompute()
    # Iteratively remove implied waits (one at a time, recomputing floors).
    for _round in range(2000):
        victim = None
        for i, ins in enumerate(insts):
            ws = waits(ins)
            if len(ws) < 2:
                continue
            for w in ws:
                if w.wait_mode != "sem-ge-imm" or w.wait_reg:
                    continue
                f = disp_floor(i, skip_wait=w)
                if f.get(w.ant_name, 0) >= w.wait_value:
                    victim = (i, w)
                    break
            if victim:
                break
        if victim is None:
            break
        i, w = victim
        si = insts[i].sync_info
        kept = [x for x in si.on_wait if x is not w]
        insts[i].sync_info = mybir.SyncInfo(on_wait=kept, on_update=si.on_update)
        for d in disp:
            d.clear()
        for d in done:
            d.clear()
        recompute()

    bad = [
        (type(ins).__name__, [(w.ant_name, w.wait_value) for w in waits(ins)])
        for ins in insts
        if len(waits(ins)) >= 2
    ]
    if bad:
        raise RuntimeError(f"instructions still carry >=2 waits: {bad[:5]}")


def round_tf32(x):
    """Round-to-nearest-even fp32 -> tf32 (10-bit mantissa), as float32 bits."""
    u = np.asarray(x, dtype=np.float32).view(np.uint32)
    bias = np.uint32(0x0FFF) + ((u >> np.uint32(13)) & np.uint32(1))
    return ((u + bias) & np.uint32(0xFFFFE000)).view(np.float32)


def make_in_maps(ctrl_tokens, prev_trace, embed):
    idx = np.asarray(ctrl_tokens)[:, :, 1].astype(np.float32)  # [B, T]
    prev = np.asarray(prev_trace, dtype=np.float32)  # [B, D]
    emb = round_tf32(np.asarray(embed, dtype=np.float32))  # [V, D]
    iota = np.arange(V, dtype=np.float32)
    apow_p = (ALPHA ** (np.arange(BLK, dtype=np.float64) + 1.0)).astype(np.float32)
    rhs = np.concatenate([emb, emb], axis=0)  # [128, D]
    in_maps = []
    for c in range(NCORES):
        rows = [RPC * c + r for r in range(RPC)]
        comb = np.zeros((128, 2 + T), np.float32)
        for r, b in enumerate(rows):
            comb[r * V : (r + 1) * V, 0] = iota
            comb[r * V : (r + 1) * V, 2:] = idx[b][None, :]
        comb[:, 1] = ALPHA
        m = {
            "comb": comb,
            "rhs": rhs,
            "apow": apow_p.reshape(128, 1).copy(),
        }
        for r, b in enumerate(rows):
            m[f"prev{r}"] = np.repeat(prev[b][None, :], 128, axis=0)
        in_maps.append(m)
    return in_maps


_NC_CACHE = None


def get_nc():
    global _NC_CACHE
    if _NC_CACHE is None:
        _NC_CACHE = build_nc()
    return _NC_CACHE


def kernel(ctrl_tokens, prev_trace, embed):
    in_maps = make_in_maps(ctrl_tokens, prev_trace, embed)
    res = run_bass_kernel_spmd(get_nc(), in_maps, core_ids=list(range(NCORES)))
    out = np.concatenate([r["out"] for r in res.results], axis=0)  # [B, T, D]
    return np.ascontiguousarray(out.astype(np.float32))


# revision 34
# speedup vs baseline: 1.1473x; 1.1473x over previous
"""EventTrace kernel for Trainium2 (8 NeuronCores, Bass/Tile).

Computes, for each batch row b:
    ev[t]   = embed[ctrl_tokens[b, t, 1]]          (gather from [64,512] table)
    c[t]    = ALPHA * c[t-1] + ev[t],  c[-1] = prev_trace[b]
    out[b]  = c                                     -> [B, T, D] float32

Algorithm (per core, 2 batch rows):
  Instead of gathering 16 MiB of embeddings, scan *decayed one-hot counts*
  G[v, t] = ALPHA * G[v, t-1] + onehot(idx_t == v) on the vector engine
  (tensor_tensor_scan, both rows in one [128, T] scan), then reconstruct
  each 128-step output block with one K=64 matmul per row:
      C[t, d] = sum_v G[v, t] * embed[v, d]  (+ ALPHA^(t+1) * prev[d])
  The two rows' matmuls use PE row-tiling (tile_position (0,0) / (64,0)) so
  they run concurrently.  The prev-trace carry decays below f32 relevance
  after 128 steps, so it is applied only to block 0 via a fused
  scalar_tensor_tensor during PSUM eviction.

Sharding: batch rows across the 8 cores (2 rows per core); the embedding
table and constants are replicated.
"""

import sys

for _p in ("/root/.axon_site/_ro/trn_rl_repo", "/opt/trn_rl_repo"):
    if _p not in sys.path:
        sys.path.append(_p)

import numpy as np

import concourse.bass as bass
import concourse.tile as tile
from concourse import mybir
from concourse.bass_utils import run_bass_kernel_spmd

ALPHA = 0.9
B, T, V, D = 16, 4096, 64, 512
NCORES = 8
RPC = B // NCORES  # batch rows per core
BLK = 128
NBLK = T // BLK
NCH = 4  # scan/pipeline chunks
TC = T // NCH
BPC = TC // BLK  # blocks per chunk

F32 = mybir.dt.float32
F32R = mybir.dt.float32r
BF16 = mybir.dt.bfloat16

# which engine evicts PSUM for block k (DVE is ~2x faster per copy but also
# runs the scan; ACT is otherwise idle and can trigger its own out-DMA)
def _copy_engine(k):
    return "act" if k % 2 == 0 else "dve"


def build_nc(strip=True):
    nc = bass.Bass(trn_type="TRN2", target_bir_lowering=False)

    # idx[b] broadcast across partitions b*64..(b+1)*64, bf16 (values 0..63)
    idx_d = nc.dram_tensor("idxin", [128, T], BF16, kind="ExternalInput")
    hdr_i = nc.dram_tensor("hdr_i", [128, 1], F32, kind="ExternalInput")  # iota
    hdr_a = nc.dram_tensor("hdr_a", [128, 1], F32, kind="ExternalInput")  # alpha
    # embed duplicated into both partition halves (pre-rounded to tf32)
    rhs_d = nc.dram_tensor("rhs", [128, D], F32, kind="ExternalInput")
    # prev_trace[b] broadcast across 128 partitions, one per row
    prev_d = [
        nc.dram_tensor(f"prev{b}", [128, D], F32, kind="ExternalInput")
        for b in range(RPC)
    ]
    # alpha^(p+1) per partition, for the block-0 carry
    apow_d = nc.dram_tensor("apow", [128, 1], F32, kind="ExternalInput")
    out = nc.dram_tensor("out", [RPC, T, D], F32, kind="ExternalOutput")

    with tile.TileContext(nc) as tc:
        with (
            tc.tile_pool(name="const", bufs=1) as cpool,
            tc.tile_pool(name="psum", bufs=8, space="PSUM") as ppool,
            tc.tile_pool(name="outp", bufs=8) as opool,
        ):
            idx_t = cpool.tile([128, T], BF16, name="idx_t")
            hdr_i_t = cpool.tile([128, 1], F32, name="hdr_i_t")
            nc.gpsimd.dma_start(hdr_i_t[:], hdr_i[:, :])
            hdr_a_t = cpool.tile([128, 1], F32, name="hdr_a_t")
            nc.gpsimd.dma_start(hdr_a_t[:], hdr_a[:, :])
            # rhs flows through a DVE cast: walrus only accepts compute-engine
            # producers for fp32r matmul operands (values must be rounded).
            rhs_stage = cpool.tile([128, D], F32, name="rhs_stage")
            nc.gpsimd.dma_start(rhs_stage[:], rhs_d[:, :])
            rhs_t = cpool.tile([128, D], F32R, name="rhs_t")
            prev_t = [
                cpool.tile([128, D], F32, name=f"prev_t{b}") for b in range(RPC)
            ]
            for b in range(RPC):
                nc.gpsimd.dma_start(prev_t[b][:], prev_d[b][:, :])
            apow_t = cpool.tile([128, 1], F32, name="apow_t")
            nc.gpsimd.dma_start(apow_t[:], apow_d[:, :])
            # per-chunk idx loads (separate DMAs so chunk 0 lands fast)
            for c in range(NCH):
                nc.gpsimd.dma_start(
                    idx_t[:, c * TC : (c + 1) * TC], idx_d[:, c * TC : (c + 1) * TC]
                )

            scr = cpool.tile([128, 8], F32, name="scr")
            nc.vector.memset(scr[:], 0.0)

            # DVE cast (also makes DVE observe the rhs DMA); the tiny copies
            # make DVE observe hdr/prev/apow so later consumers need one wait.
            nc.vector.tensor_copy(rhs_t[:], rhs_stage[:])
            nc.vector.tensor_copy(scr[0:1, 1:2], prev_t[0][0:1, 0:1])
            nc.vector.tensor_copy(scr[0:1, 2:3], prev_t[1][0:1, 0:1])
            nc.vector.tensor_copy(scr[0:1, 3:4], apow_t[0:1, 0:1])
            nc.vector.tensor_copy(scr[0:1, 4:5], hdr_a_t[0:1, 0:1])
            nc.vector.tensor_copy(scr[0:1, 5:6], hdr_i_t[0:1, 0:1])

            m2 = cpool.tile([128, T], F32, name="m2")
            g2 = cpool.tile([128, T], F32R, name="g2")

            def scan_chunk(c):
                cs, ce = c * TC, (c + 1) * TC
                # M[p, t] = 1.0 if idx[p//64, t] == (p % 64) else 0.0
                nc.vector.tensor_scalar(
                    m2[:, cs:ce],
                    idx_t[:, cs:ce],
                    hdr_i_t[:, 0:1],
                    None,
                    mybir.AluOpType.is_equal,
                )
                # G[p, t] = ALPHA * G[p, t-1] + M[p, t]   (both rows at once)
                nc.vector.tensor_tensor_scan(
                    g2[:, cs:ce],
                    hdr_a_t[:, 0:1].broadcast_to((128, TC)),
                    m2[:, cs:ce],
                    0.0 if c == 0 else g2[:, cs - 1 : cs],
                    mybir.AluOpType.mult,
                    mybir.AluOpType.add,
                )

            last_ots = []
            scan_chunk(0)
            for c in range(NCH):
                if c + 1 < NCH:
                    scan_chunk(c + 1)
                for kk in range(BPC):
                    k = c * BPC + kk
                    for b in range(RPC):
                        ps = ppool.tile([BLK, D], F32, name="ps")
                        nc.tensor.matmul(
                            ps[:],
                            g2[b * V : (b + 1) * V, k * BLK : (k + 1) * BLK],
                            rhs_t[b * V : (b + 1) * V, :],
                            start=True,
                            stop=True,
                            tile_position=(b * V, 0),
                        )
                        ot = opool.tile([BLK, D], F32, name="ot")
                        wr = "dve" if k == 0 else _copy_engine(k)
                        # 4-byte touch absorbs the WAR wait on this slot's
                        # prior out-DMA, so the eviction waits only on the MM.
                        if wr == "act":
                            nc.scalar.copy(ot[0:1, 0:1], scr[0:1, 0:1])
                        else:
                            nc.vector.tensor_copy(ot[0:1, 0:1], scr[0:1, 0:1])
                        if k == 0:
                            # block 0 carries prev: ot = prev*alpha^(p+1) + ps
                            nc.vector.scalar_tensor_tensor(
                                ot[:],
                                prev_t[b][:],
                                apow_t[:, 0:1],
                                ps[:],
                                mybir.AluOpType.mult,
                                mybir.AluOpType.add,
                            )
                        elif wr == "act":
                            nc.scalar.copy(ot[:], ps[:])
                        else:
                            nc.vector.tensor_copy(ot[:], ps[:])
                        # ACT-evicted blocks trigger their own out-DMA
                        # (same engine -> no extra wait); others via SP.
                        dma_eng = nc.scalar if wr == "act" else nc.sync
                        dma_eng.dma_start(out[b, k * BLK : (k + 1) * BLK, :], ot[:])
                        last_ots.append(ot)
                        last_ots = last_ots[-8:]
            # End-of-kernel sinks: writing each of the last 8 output slots
            # makes the DVE stream transitively observe every DMA queue's
            # final completion, so the tail drain needs only one wait after
            # the redundant-wait strip below.
            for ot in last_ots:
                nc.vector.tensor_copy(ot[0:1, 0:1], scr[0:1, 0:1])
    if strip:
        _strip_redundant_waits(nc)
    return nc


def _strip_redundant_waits(nc):
    """Remove statically-implied semaphore waits (vector-clock analysis).

    The TRN2 instruction encodings here accept only ONE sync-wait command
    per instruction, but Tile emits extra waits for pool-slot reuse and the
    kernel-tail drain.  Many of those waits are statically implied by
    program order: engine queues execute in order, each DMA queue completes
    FIFO, and observing a semaphore value inherits every guarantee its
    updaters had.  This pass computes, for every instruction, the semaphore
    floor guaranteed at issue, and drops any wait already implied without
    it.  Straight-line (loop-free) programs only.
    """
    import concourse.mybir as mybir

    insts = []
    for fn in nc.m.functions:
        for bb in fn.blocks:
            for ins in bb.instructions:
                insts.append(ins)

    def waits(ins):
        si = ins.sync_info
        return list(si.on_wait) if si is not None else []

    def updates(ins):
        si = ins.sync_info
        return list(si.on_update) if si is not None else []

    # Streams: compute instructions execute in order per engine; a DMACopy's
    # *data completion* (its sem update) is FIFO per DMA queue, gated by its
    # trigger (engine stream) issue.
    def is_dma(ins):
        return type(ins).__name__ == "InstDMACopy"

    def dma_queue(ins):
        us = updates(ins)
        return us[0].ant_name if us else None

    # sem -> ordered list of (inst_index, add_value); single-updater-stream
    # sems only are used for transitive guarantees.
    sem_updaters = {}
    sem_streams = {}
    for i, ins in enumerate(insts):
        key = ("q", dma_queue(ins)) if is_dma(ins) else ("e", str(ins.engine))
        for u in updates(ins):
            if u.update_mode not in ("sem-inc", "sem-add-imm") or u.update_reg:
                sem_streams.setdefault(u.ant_name, set()).add("reg")
                continue
            sem_updaters.setdefault(u.ant_name, []).append((i, u.update_value))
            sem_streams.setdefault(u.ant_name, set()).add(key)

    single_stream_sems = {s for s, st in sem_streams.items() if len(st) == 1}

    # cumulative sem value right after instruction i's update
    cum_after = {}
    run = {}
    for i, ins in enumerate(insts):
        for u in updates(ins):
            if u.update_mode in ("sem-inc", "sem-add-imm") and not u.update_reg:
                run[u.ant_name] = run.get(u.ant_name, 0) + u.update_value
                cum_after[(i, u.ant_name)] = run[u.ant_name]

    prev_engine = {}
    prev_queue = {}
    last_e = {}
    last_q = {}
    for i, ins in enumerate(insts):
        ek = str(ins.engine)
        prev_engine[i] = last_e.get(ek)
        last_e[ek] = i
        if is_dma(ins):
            qk = dma_queue(ins)
            prev_queue[i] = last_q.get(qk)
            last_q[qk] = i

    n = len(insts)
    # disp[i]: sem floor guaranteed when instruction i dispatches (data-order
    # level).  done[i]: floor when its effects (sem updates) are visible —
    # for a DMACopy that is DATA completion on its queue.
    disp = [dict() for _ in range(n)]
    done = [dict() for _ in range(n)]

    def join_into(dst, src):
        changed = False
        for s, v in src.items():
            if dst.get(s, 0) < v:
                dst[s] = v
                changed = True
        return changed

    def guarantee_of_wait(sem, val):
        """Floor implied by observing sem >= val."""
        out = {sem: val}
        if sem not in single_stream_sems:
            return out
        cum = 0
        for j, add in sem_updaters.get(sem, []):
            cum += add
            join_into(out, done[j])
            if cum >= val:
                break
        return out

    def disp_floor(i, skip_wait=None):
        out = {}
        p = prev_engine[i]
        if p is not None:
            join_into(out, disp[p])
            if not is_dma(insts[p]):
                # same-engine execution is in-order: p's effects precede i's
                join_into(out, done[p])
        for w in waits(insts[i]):
            if w is skip_wait:
                continue
            if w.wait_mode == "sem-ge-imm" and not w.wait_reg:
                join_into(out, guarantee_of_wait(w.ant_name, w.wait_value))
        return out

    def recompute():
        changed = True
        while changed:
            changed = False
            for i, ins in enumerate(insts):
                f = disp_floor(i)
                if join_into(disp[i], f):
                    changed = True
                d = dict(disp[i])
                if is_dma(ins):
                    pq = prev_queue.get(i)
                    if pq is not None:
                        join_into(d, done[pq])
                for u in updates(ins):
                    c = cum_after.get((i, u.ant_name))
                    if c is not None and d.get(u.ant_name, 0) < c:
                        d[u.ant_name] = c
                if join_into(done[i], d):
                    changed = True

    recompute()
    # Iteratively remove implied waits (one at a time, recomputing floors).
    for _round in range(2000):
        victim = None
        for i, ins in enumerate(insts):
            ws = waits(ins)
            if len(ws) < 2:
                continue
            for w in ws:
                if w.wait_mode != "sem-ge-imm" or w.wait_reg:
                    continue
                # A DMA trigger's wait on its OWN queue's semaphore is ring
                # backpressure, not a data dependency: same-queue DMAs
                # complete FIFO regardless, and this kernel keeps well under
                # the HWDGE ring depth per queue.  Droppable.
                if is_dma(ins) and w.ant_name == dma_queue(ins):
                    victim = (i, w)
                    break
                f = disp_floor(i, skip_wait=w)
                if f.get(w.ant_name, 0) >= w.wait_value:
                    victim = (i, w)
                    break
            if victim:
                break
        if victim is None:
            break
        i, w = victim
        si = insts[i].sync_info
        kept = [x for x in si.on_wait if x is not w]
        insts[i].sync_info = mybir.SyncInfo(on_wait=kept, on_update=si.on_update)
        for d in disp:
            d.clear()
        for d in done:
            d.clear()
        recompute()

    bad = [
        (type(ins).__name__, [(w.ant_name, w.wait_value) for w in waits(ins)])
        for ins in insts
        if len(waits(ins)) >= 2
    ]
    if bad:
        raise RuntimeError(f"instructions still carry >=2 waits: {bad[:5]}")


def round_tf32(x):
    """Round-to-nearest-even fp32 -> tf32 (10-bit mantissa), as float32 bits."""
    u = np.asarray(x, dtype=np.float32).view(np.uint32)
    bias = np.uint32(0x0FFF) + ((u >> np.uint32(13)) & np.uint32(1))
    return ((u + bias) & np.uint32(0xFFFFE000)).view(np.float32)


def make_in_maps(ctrl_tokens, prev_trace, embed):
    import ml_dtypes

    bf16 = ml_dtypes.bfloat16
    idx = np.asarray(ctrl_tokens)[:, :, 1].astype(bf16)  # [B, T] (values < 64)
    prev = np.asarray(prev_trace, dtype=np.float32)  # [B, D]
    emb = round_tf32(np.asarray(embed, dtype=np.float32))  # [V, D]
    iota = np.arange(V, dtype=np.float32)
    apow_p = (ALPHA ** (np.arange(BLK, dtype=np.float64) + 1.0)).astype(np.float32)
    rhs = np.concatenate([emb, emb], axis=0)  # [128, D]
    hdr_i = np.concatenate([iota, iota]).reshape(128, 1).copy()
    hdr_a = np.full((128, 1), ALPHA, np.float32)
    in_maps = []
    for c in range(NCORES):
        rows = [RPC * c + r for r in range(RPC)]
        idxin = np.empty((128, T), bf16)
        for r, b in enumerate(rows):
            idxin[r * V : (r + 1) * V, :] = idx[b][None, :]
        m = {
            "idxin": idxin,
            "hdr_i": hdr_i,
            "hdr_a": hdr_a,
            "rhs": rhs,
            "apow": apow_p.reshape(128, 1).copy(),
        }
        for r, b in enumerate(rows):
            m[f"prev{r}"] = np.repeat(prev[b][None, :], 128, axis=0)
        in_maps.append(m)
    return in_maps


_NC_CACHE = None


def get_nc():
    global _NC_CACHE
    if _NC_CACHE is None:
        _NC_CACHE = build_nc()
    return _NC_CACHE


def kernel(ctrl_tokens, prev_trace, embed):
    in_maps = make_in_maps(ctrl_tokens, prev_trace, embed)
    res = run_bass_kernel_spmd(get_nc(), in_maps, core_ids=list(range(NCORES)))
    out = np.concatenate([r["out"] for r in res.results], axis=0)  # [B, T, D]
    return np.ascontiguousarray(out.astype(np.float32))
